# revision 1
# baseline (speedup 1.0000x reference)
"""Trainium2 Bass kernel for nn_NeighborAggregator (GNN message passing).

A_raw[i] = sum_e [adj_rows[e]==i] * adj_values[e] * x[adj_rows[e], adj_cols[e]]
alpha    = softmax(A_raw)
returns (alpha, A_raw)

Strategy (8 NeuronCores):
  - Shard rows of x across cores (1024 rows each).
  - Host scatters adj_values into a dense per-core mask W (rows x 8192 f32);
    device streams x-shard and W through a fused DVE tensor_tensor_reduce
    (multiply + row-sum) per 128-row tile -> per-core A_local (1024).
  - AllGather A_local across the 8 cores -> full A_raw (8192) on every core.
  - On-device softmax (global max via PE transpose + K=1 matmul broadcast,
    exp on ACT with fused row-sums, global sum via ones-matmul broadcast).
"""
import numpy as np
from contextlib import ExitStack

import concourse.tile as tile
from concourse import bass, bacc, mybir
from concourse.bass_utils import run_bass_kernel_spmd

N = 8192
E = 524288
NCORES = 8
RPC = N // NCORES          # rows per core = 1024
P = 128
NTILES = RPC // P          # 8 row-tiles per core

_cache = {}


def _build():
    nc = bacc.Bacc(None)
    x = nc.dram_tensor("x", [NTILES, P, N], mybir.dt.float32,
                       kind="ExternalInput")
    w = nc.dram_tensor("w", [NTILES, P, N], mybir.dt.float32,
                       kind="ExternalInput")
    alpha_out = nc.dram_tensor("alpha", [N], mybir.dt.float32,
                               kind="ExternalOutput")
    araw_out = nc.dram_tensor("araw", [N], mybir.dt.float32,
                              kind="ExternalOutput")

    with tile.TileContext(nc) as tc:
        with ExitStack() as ctx:
            sbuf = ctx.enter_context(tc.tile_pool(name="sbuf", bufs=2))
            one = ctx.enter_context(tc.tile_pool(name="one", bufs=1))
            psum = ctx.enter_context(
                tc.tile_pool(name="psum", bufs=1, space="PSUM"))
            dram = ctx.enter_context(
                tc.tile_pool(name="dram", bufs=1, space="DRAM"))

            a_cols = one.tile([P, NTILES], mybir.dt.float32)
            cc_in = dram.tile([RPC, 1], mybir.dt.float32)
            cc_out0 = dram.tile([N // 2, 1], mybir.dt.float32,
                                addr_space="Shared")
            cc_out1 = dram.tile([N // 2, 1], mybir.dt.float32,
                                addr_space="Shared")
            HT = NTILES // 2
            for t in range(NTILES):
                x_t = sbuf.tile([P, N], mybir.dt.float32)
                w_t = sbuf.tile([P, N], mybir.dt.float32)
                nc.sync.dma_start(out=x_t[:], in_=x[t])
                nc.sync.dma_start(out=w_t[:], in_=w[t])
                nc.vector.tensor_tensor(out=w_t[:], in0=x_t[:], in1=w_t[:],
                                        op=mybir.AluOpType.mult)
                nc.vector.tensor_reduce(out=a_cols[:, t:t + 1], in_=w_t[:],
                                        axis=mybir.AxisListType.X,
                                        op=mybir.AluOpType.add)
                nc.scalar.dma_start(out=cc_in[t * P:(t + 1) * P, :],
                                    in_=a_cols[:, t:t + 1])
                if t == HT - 1:
                    # first-half AllGather overlaps tiles HT..NTILES-1
                    nc.gpsimd.collective_compute(
                        "AllGather", mybir.AluOpType.bypass,
                        replica_groups=[list(range(NCORES))],
                        ins=[cc_in[:HT * P]], outs=[cc_out0[:]])
            nc.gpsimd.collective_compute(
                "AllGather", mybir.AluOpType.bypass,
                replica_groups=[list(range(NCORES))],
                ins=[cc_in[HT * P:]], outs=[cc_out1[:]])

            # reassemble true row order: cc_out{0,1}[c*512+j] = A[c*1024+{0,512}+j]
            araw_v = araw_out[:].rearrange("(c j) -> c j", c=NCORES)
            nc.sync.dma_start(out=araw_v[:, :RPC // 2],
                              in_=cc_out0[:, 0].rearrange("(c j) -> c j",
                                                          c=NCORES))
            nc.sync.dma_start(out=araw_v[:, RPC // 2:],
                              in_=cc_out1[:, 0].rearrange("(c j) -> c j",
                                                          c=NCORES))

            # ---- softmax over the full 8192 vector ----
            F = N // P   # 64
            af = one.tile([P, F], mybir.dt.float32)
            nc.sync.dma_start(
                out=af[:], in_=araw_out[:].rearrange("(p f) -> p f", p=P))

            # global max -> negated, broadcast to all partitions
            m = one.tile([P, 1], mybir.dt.float32)
            nc.vector.tensor_reduce(out=m[:], in_=af[:],
                                    axis=mybir.AxisListType.X,
                                    op=mybir.AluOpType.max)
            ident = one.tile([P, P], mybir.dt.float32)
            from concourse.masks import make_identity
            make_identity(nc, ident[:])
            mt_ps = psum.tile([P, P], mybir.dt.float32, space="PSUM")
            nc.tensor.transpose(out=mt_ps[:1, :], in_=m[:, :1], identity=ident[:])
            mt = one.tile([1, P], mybir.dt.float32)
            nc.vector.tensor_copy(out=mt[:], in_=mt_ps[:1, :])
            gmax = one.tile([1, 1], mybir.dt.float32)
            nc.vector.tensor_reduce(out=gmax[:], in_=mt[:],
                                    axis=mybir.AxisListType.X,
                                    op=mybir.AluOpType.max)
            ngmax = one.tile([1, 1], mybir.dt.float32)
            nc.vector.tensor_scalar(out=ngmax[:], in0=gmax[:],
                                    scalar1=-1.0, scalar2=None,
                                    op0=mybir.AluOpType.mult)
            ones_row = one.tile([1, P], mybir.dt.float32)
            nc.vector.memset(ones_row[:], 1.0)
            nb_ps = psum.tile([P, 1], mybir.dt.float32, space="PSUM")
            nc.tensor.matmul(out=nb_ps[:], lhsT=ones_row[:], rhs=ngmax[:],
                             start=True, stop=True)
            nbias = one.tile([P, 1], mybir.dt.float32)
            nc.vector.tensor_copy(out=nbias[:], in_=nb_ps[:])

            # e = exp(af - gmax), rowsum fused
            ex = one.tile([P, F], mybir.dt.float32)
            rsum = one.tile([P, 1], mybir.dt.float32)
            nc.scalar.activation(out=ex[:], in_=af[:],
                                 func=mybir.ActivationFunctionType.Exp,
                                 bias=nbias[:, :1], scale=1.0,
                                 accum_out=rsum[:])

            # total = sum over all partitions (ones-matmul broadcast)
            ones_sq = one.tile([P, P], mybir.dt.float32)
            nc.vector.memset(ones_sq[:], 1.0)
            tot_ps = psum.tile([P, 1], mybir.dt.float32, space="PSUM")
            nc.tensor.matmul(out=tot_ps[:], lhsT=ones_sq[:], rhs=rsum[:],
                             start=True, stop=True)
            rtot = one.tile([P, 1], mybir.dt.float32)
            nc.vector.reciprocal(out=rtot[:], in_=tot_ps[:])

            alpha_t = one.tile([P, F], mybir.dt.float32)
            nc.vector.tensor_tensor(out=alpha_t[:], in0=ex[:],
                                    in1=rtot[:].to_broadcast([P, F]),
                                    op=mybir.AluOpType.mult)
            nc.sync.dma_start(
                out=alpha_out[:].rearrange("(p f) -> p f", p=P),
                in_=alpha_t[:])
    nc.compile()
    return nc


def _host_shards(data_input, adj_values, adj_rows, adj_cols):
    x = np.ascontiguousarray(np.asarray(data_input, dtype=np.float32)[0])
    v = np.asarray(adj_values, dtype=np.float32)
    r = np.asarray(adj_rows, dtype=np.int64)
    c = np.asarray(adj_cols, dtype=np.int64)
    in_maps = []
    for k in range(NCORES):
        lo = k * RPC
        sel = (r >= lo) & (r < lo + RPC)
        rl = r[sel] - lo
        cl = c[sel]
        vl = v[sel].astype(np.float64)
        flat = rl * N + cl
        wk = np.bincount(flat, weights=vl, minlength=RPC * N)
        wk = wk.astype(np.float32).reshape(NTILES, P, N)
        xk = x[lo:lo + RPC].reshape(NTILES, P, N)
        in_maps.append({"x": xk, "w": wk})
    return in_maps


def kernel(data_input, adj_values, adj_rows, adj_cols):
    if "nc" not in _cache:
        _cache["nc"] = _build()
    nc = _cache["nc"]
    in_maps = _host_shards(data_input, adj_values, adj_rows, adj_cols)
    res = run_bass_kernel_spmd(nc, in_maps, list(range(NCORES)))
    alpha = res.results[0]["alpha"].reshape(N).astype(np.float32)
    araw = res.results[0]["araw"].reshape(N).astype(np.float32)
    return (alpha, araw)

